# revision 1
# baseline (speedup 1.0000x reference)
"""2-layer GAT (PyG GATConv, heads=1) on 8 Trainium2 NeuronCores.

Strategy (dst-owner sharding, per spec sharding_hint):
  - Nodes split into 8 contiguous chunks of N/8; edges owned by dst's core.
  - 3 NEFF launches (host does only data movement between them):
    NEFF#1: per-core h1 = embed_chunk @ W1, s1/d1 = h1 @ a_{src,dst}1,
            emitted as bf16 hi/lo split rows -> host concats into a full
            gather table T1 [N, 384]bf16 = [h_hi|h_lo|1.0|s_hi|s_lo|pad].
    NEFF#2: L1 edge phase per core: dma_gather T1 rows by edge src,
            attention weights w_e = exp(LeakyReLU(s_src + d_dst)) computed
            via the separable form  w = max(exp(s)exp(d), exp(.2s)exp(.2d)),
            folded into a per-128-edge-group indicator matrix
            S_alpha[e, c] = w_e * 1[dstcol_e == c]  (one fused DVE op),
            aggregated on the TensorEngine: psum += S_alpha^T @ [h|1].
            The trailing ones column yields the softmax denominator Z free.
            Tail: x2 = relu(out1+b1); h2 = x2 @ W2; s2/d2 -> T2 chunks.
    NEFF#3: same edge machinery on T2 [N, 256]bf16, final sigmoid.
  - Edges are bucketed into <=32768-row source "sets" (dma_gather idx is
    int16) and into 127-dst psum windows; group counts G[set][window] are
    maxed across cores so all 8 cores run one SPMD instruction stream.
"""
import sys

if '/opt/trn_rl_repo' not in sys.path:
    sys.path.insert(0, '/opt/trn_rl_repo')

import numpy as np
import ml_dtypes

from concourse import bacc, mybir
import concourse.tile as tile
from concourse.bass_utils import run_bass_kernel_spmd
from concourse.masks import make_identity

BF16 = ml_dtypes.bfloat16
NCORES = 8
WIN = 127          # dsts per psum window (col 127 = dummy slot)
MW = 2             # windows per gather megatile
SETROWS = 32768    # int16 gather index range
F32 = mybir.dt.float32
BF = mybir.dt.bfloat16
I16 = mybir.dt.int16
AF = mybir.ActivationFunctionType
OP = mybir.AluOpType


# ----------------------------------------------------------------- host pre
def _preprocess(edge_index, N):
    CH = N // NCORES
    NW = -(-CH // WIN)
    NS = -(-N // SETROWS)
    src = np.concatenate([edge_index[0], np.arange(N, dtype=np.int64)])
    dst = np.concatenate([edge_index[1], np.arange(N, dtype=np.int64)])
    src = src.astype(np.int64)
    dst = dst.astype(np.int64)
    owner = dst // CH
    dl = dst - owner * CH
    sid = src // SETROWS

    cnt = np.zeros((NCORES, NS, NW), np.int64)
    percs = []
    for c in range(NCORES):
        mc = owner == c
        percs.append((src[mc], dl[mc], sid[mc]))
        for s in range(NS):
            ms = percs[c][2] == s
            w = percs[c][1][ms] // WIN
            cnt[c, s] = np.bincount(w, minlength=NW)
    G = -(-cnt.max(axis=0) // 128)          # [NS, NW] groups per (set, window)
    G[cnt.max(axis=0) == 0] = 0

    cumG = np.zeros((NS, NW + 1), np.int64)
    cumG[:, 1:] = np.cumsum(G, axis=1)
    nslot = 128 * cumG[:, -1]               # per-set stream length

    cores = []
    for c in range(NCORES):
        csrc, cdl, csid = percs[c]
        gidx, dcol = [], []
        for s in range(NS):
            ms = csid == s
            esrc, edl = csrc[ms], cdl[ms]
            order = np.argsort(edl, kind='stable')
            esrc, edl = esrc[order], edl[order]
            w = edl // WIN
            col = edl - w * WIN
            # rank within window
            cc = np.zeros(NW + 1, np.int64)
            cc[1:] = np.cumsum(np.bincount(w, minlength=NW))
            rank = np.arange(len(edl)) - cc[w]
            slot = 128 * cumG[s][w] + rank
            arr_i = np.zeros(nslot[s], np.int16)
            arr_c = np.full(nslot[s], 127.0, np.float32)
            arr_i[slot] = (esrc - s * SETROWS).astype(np.int16)
            arr_c[slot] = col
            gi = np.tile(arr_i.reshape(-1, 16).T, (8, 1)) if nslot[s] else \
                np.zeros((128, 0), np.int16)
            dc = arr_c.reshape(-1, 128).T.astype(BF16) if nslot[s] else \
                np.zeros((128, 0), BF16)
            gidx.append(np.ascontiguousarray(gi))
            dcol.append(np.ascontiguousarray(dc))
        cores.append((gidx, dcol))
    return dict(CH=CH, NW=NW, NS=NS, G=G, cumG=cumG, nslot=nslot, cores=cores)


# ------------------------------------------------------------------ NEFF #1
def _build_neff1(N, C, H, CH):
    nc = bacc.Bacc(None, target_bir_lowering=False)
    xT = nc.declare_dram_parameter("xT", [C, CH], F32, isOutput=False)
    W1 = nc.declare_dram_parameter("W1", [C, H], F32, isOutput=False)
    a1s = nc.declare_dram_parameter("a1s", [H, 1], F32, isOutput=False)
    a1d = nc.declare_dram_parameter("a1d", [H, 1], F32, isOutput=False)
    hhi = nc.declare_dram_parameter("hhi", [H, CH], BF, isOutput=True)
    hlo = nc.declare_dram_parameter("hlo", [H, CH], BF, isOutput=True)
    shi = nc.declare_dram_parameter("shi", [1, CH], BF, isOutput=True)
    slo = nc.declare_dram_parameter("slo", [1, CH], BF, isOutput=True)
    d1o = nc.declare_dram_parameter("d1o", [1, CH], F32, isOutput=True)

    KT = -(-C // 128)
    with tile.TileContext(nc) as tc:
        with tc.tile_pool(name="cst", bufs=1) as cp, \
             tc.tile_pool(name="wk", bufs=3) as wp, \
             tc.tile_pool(name="ps", bufs=2, space="PSUM") as pp, \
             tc.tile_pool(name="ps1", bufs=2, space="PSUM") as pp1:
            xts, w1s = [], []
            for k in range(KT):
                kc = min(128, C - 128 * k)
                xt = cp.tile([kc, CH], F32, tag=f"xt{k}")
                nc.sync.dma_start(out=xt[:], in_=xT[128 * k:128 * k + kc, :])
                w1 = cp.tile([kc, H], F32, tag=f"w1{k}")
                nc.sync.dma_start(out=w1[:], in_=W1[128 * k:128 * k + kc, :])
                xts.append(xt)
                w1s.append(w1)
            asb = cp.tile([H, 1], F32, tag="a1s")
            nc.sync.dma_start(out=asb[:], in_=a1s[:])
            adb = cp.tile([H, 1], F32, tag="a1d")
            nc.sync.dma_start(out=adb[:], in_=a1d[:])
            h1T = cp.tile([H, CH], F32, tag="h1T")

            CW = 500
            for o in range(0, CH, CW):
                cw = min(CW, CH - o)
                ph = pp.tile([H, CW], F32, space="PSUM", tag="ph")
                for k in range(KT):
                    nc.tensor.matmul(out=ph[:, :cw], lhsT=w1s[k][:],
                                     rhs=xts[k][:, o:o + cw],
                                     start=(k == 0), stop=(k == KT - 1))
                nc.vector.tensor_copy(out=h1T[:, o:o + cw], in_=ph[:, :cw])
                hh = wp.tile([H, CW], BF, tag="hh")
                nc.scalar.activation(hh[:, :cw], ph[:, :cw], AF.Copy)
                tmp = wp.tile([H, CW], F32, tag="tmp")
                nc.vector.tensor_tensor(out=tmp[:, :cw], in0=ph[:, :cw],
                                        in1=hh[:, :cw], op=OP.subtract)
                hl = wp.tile([H, CW], BF, tag="hl")
                nc.vector.tensor_copy(out=hl[:, :cw], in_=tmp[:, :cw])
                nc.sync.dma_start(out=hhi[:, o:o + cw], in_=hh[:, :cw])
                nc.sync.dma_start(out=hlo[:, o:o + cw], in_=hl[:, :cw])
            for o in range(0, CH, CW):
                cw = min(CW, CH - o)
                ps = pp1.tile([1, CW], F32, space="PSUM", tag="psv")
                nc.tensor.matmul(out=ps[:, :cw], lhsT=asb[:],
                                 rhs=h1T[:, o:o + cw], start=True, stop=True)
                sh = wp.tile([1, CW], BF, tag="sh")
                nc.scalar.activation(sh[:, :cw], ps[:, :cw], AF.Copy)
                tmp = wp.tile([1, CW], F32, tag="tms")
                nc.vector.tensor_tensor(out=tmp[:, :cw], in0=ps[:, :cw],
                                        in1=sh[:, :cw], op=OP.subtract)
                sl = wp.tile([1, CW], BF, tag="sl")
                nc.vector.tensor_copy(out=sl[:, :cw], in_=tmp[:, :cw])
                nc.sync.dma_start(out=shi[:, o:o + cw], in_=sh[:, :cw])
                nc.sync.dma_start(out=slo[:, o:o + cw], in_=sl[:, :cw])
                pd = pp1.tile([1, CW], F32, space="PSUM", tag="pdv")
                nc.tensor.matmul(out=pd[:, :cw], lhsT=adb[:],
                                 rhs=h1T[:, o:o + cw], start=True, stop=True)
                dv = wp.tile([1, CW], F32, tag="dv")
                nc.vector.tensor_copy(out=dv[:, :cw], in_=pd[:, :cw])
                nc.sync.dma_start(out=d1o[:, o:o + cw], in_=dv[:, :cw])
    nc.finalize()
    return nc


# --------------------------------------------------------- edge-phase NEFFs
def _build_edge_neff(N, CH, NW, NS, G, cumG, nslot, layer, FH, FO, Hnext):
    """layer 1: aggregates FH-dim messages, computes x2=relu(.+b1), h2/s2/d2.
       layer 2: aggregates FH-dim messages, emits sigmoid output [CH, FH].
       FH: feature dim of this layer's h.  FO: next-layer dim (layer 1 only).
    """
    TC = 384 if layer == 1 else 256
    SC = 2 * FH + 1                     # s_hi col (after h_hi, h_lo, ones)
    RC = 2 * FH + 1                     # rhs cols: h_hi | h_lo | ones
    WT = NW * WIN
    BW = WT + 128                       # padded width for B/D slices

    nc = bacc.Bacc(None, target_bir_lowering=False)
    T = nc.declare_dram_parameter("T", [N, TC], BF, isOutput=False)
    dloc = nc.declare_dram_parameter("dloc", [1, BW], F32, isOutput=False)
    iot = nc.declare_dram_parameter("iot", [128, 128], BF, isOutput=False)
    one1 = nc.declare_dram_parameter("one1", [1, 128], BF, isOutput=False)
    brep = nc.declare_dram_parameter("brep", [128, FH], F32, isOutput=False)
    gidx_d, dcol_d = [], []
    for s in range(NS):
        if nslot[s] == 0:
            gidx_d.append(None)
            dcol_d.append(None)
            continue
        gidx_d.append(nc.declare_dram_parameter(
            f"gidx{s}", [128, nslot[s] // 16], I16, isOutput=False))
        dcol_d.append(nc.declare_dram_parameter(
            f"dcol{s}", [128, nslot[s] // 128], BF, isOutput=False))
    if layer == 1:
        W2 = nc.declare_dram_parameter("W2", [FH, FO], F32, isOutput=False)
        a2s = nc.declare_dram_parameter("a2s", [FO, 1], F32, isOutput=False)
        a2d = nc.declare_dram_parameter("a2d", [FO, 1], F32, isOutput=False)
        hhi = nc.declare_dram_parameter("hhi", [FO, WT], BF, isOutput=True)
        hlo = nc.declare_dram_parameter("hlo", [FO, WT], BF, isOutput=True)
        shi = nc.declare_dram_parameter("shi", [1, WT], BF, isOutput=True)
        slo = nc.declare_dram_parameter("slo", [1, WT], BF, isOutput=True)
        d2o = nc.declare_dram_parameter("d2o", [1, WT], F32, isOutput=True)
    else:
        outp = nc.declare_dram_parameter("out", [CH, FH], F32, isOutput=True)

    # megatile group spans per set
    mts = []
    for wa in range(0, NW, MW):
        wb = min(wa + MW, NW)
        span = [(int(cumG[s][wa]), int(cumG[s][wb])) for s in range(NS)]
        mts.append((wa, wb, span))
    maxg = [max((b - a) for _, _, sp in mts for (a, b) in [sp[s]]) or 1
            for s in range(NS)]

    with tile.TileContext(nc) as tc:
        with tc.tile_pool(name="cst", bufs=1) as cp:
            iosb = cp.tile([128, 128], BF, tag="io")
            nc.sync.dma_start(out=iosb[:], in_=iot[:])
            onsb = cp.tile([1, 128], BF, tag="on")
            nc.sync.dma_start(out=onsb[:], in_=one1[:])
            bsb = cp.tile([128, FH], F32, tag="bs")
            nc.sync.dma_start(out=bsb[:], in_=brep[:])
            Bt = cp.tile([1, BW], BF, tag="Bt")
            Dt = cp.tile([1, BW], BF, tag="Dt")
            with tc.tile_pool(name="dtmp", bufs=1) as dtp:
                dsb = dtp.tile([1, BW], F32, tag="ds")
                nc.sync.dma_start(out=dsb[:], in_=dloc[:])
                nc.scalar.activation(Bt[:], dsb[:], AF.Exp)
                nc.scalar.activation(Dt[:], dsb[:], AF.Exp, scale=0.2)
            if layer == 1:
                idn = cp.tile([128, 128], F32, tag="idn")
                make_identity(nc, idn[:])
                x2T = cp.tile([128, WT], F32, tag="x2T")
                w2sb = cp.tile([FH, FO], F32, tag="w2")
                nc.sync.dma_start(out=w2sb[:], in_=W2[:])
                a2ssb = cp.tile([FO, 1], F32, tag="a2s")
                nc.sync.dma_start(out=a2ssb[:], in_=a2s[:])
                a2dsb = cp.tile([FO, 1], F32, tag="a2d")
                nc.sync.dma_start(out=a2dsb[:], in_=a2d[:])

            with tc.tile_pool(name="gth", bufs=2) as gp, \
                 tc.tile_pool(name="wk", bufs=4) as wp, \
                 tc.tile_pool(name="msk", bufs=4) as mp, \
                 tc.tile_pool(name="pm", bufs=2, space="PSUM") as pmp, \
                 tc.tile_pool(name="pb", bufs=2, space="PSUM") as pbp, \
                 tc.tile_pool(name="pt", bufs=2, space="PSUM") as ptp:
                for wa, wb, span in mts:
                    gts, Ats, Cts, dcs = [], [], [], []
                    for s in range(NS):
                        ga, gb = span[s]
                        if gb == ga:
                            gts.append(None)
                            Ats.append(None)
                            Cts.append(None)
                            dcs.append(None)
                            continue
                        gsp = gb - ga
                        ix = gp.tile([128, maxg[s] * 8], I16, tag=f"ix{s}")
                        nc.sync.dma_start(out=ix[:, :gsp * 8],
                                          in_=gidx_d[s][:, ga * 8:gb * 8])
                        gt = gp.tile([128, maxg[s], TC], BF, tag=f"gt{s}")
                        nc.gpsimd.dma_gather(
                            out_ap=gt[:, :gsp, :],
                            in_ap=T[s * SETROWS:, :],
                            idxs_ap=ix[:, :gsp * 8],
                            num_idxs=gsp * 128,
                            num_idxs_reg=gsp * 128,
                            elem_size=TC,
                            single_packet=False,
                        )
                        dc = wp.tile([128, maxg[s]], BF, tag=f"dc{s}")
                        nc.sync.dma_start(out=dc[:, :gsp],
                                          in_=dcol_d[s][:, ga:gb])
                        se = wp.tile([128, maxg[s]], F32, tag=f"se{s}")
                        nc.vector.tensor_tensor(out=se[:, :gsp],
                                                in0=gt[:, :gsp, SC],
                                                in1=gt[:, :gsp, SC + 1],
                                                op=OP.add)
                        At = wp.tile([128, maxg[s]], F32, tag=f"At{s}")
                        nc.scalar.activation(At[:, :gsp], se[:, :gsp], AF.Exp)
                        Ct = wp.tile([128, maxg[s]], F32, tag=f"Ct{s}")
                        nc.scalar.activation(Ct[:, :gsp], se[:, :gsp], AF.Exp,
                                             scale=0.2)
                        gts.append(gt)
                        Ats.append(At)
                        Cts.append(Ct)
                        dcs.append(dc)
                    for w in range(wa, wb):
                        ngrp = int(G[:, w].sum())
                        if ngrp == 0:
                            continue
                        w0 = w * WIN
                        pb = pbp.tile([128, 128], F32, space="PSUM", tag="pb")
                        nc.tensor.matmul(out=pb[:], lhsT=onsb[:],
                                         rhs=Bt[:, w0:w0 + 128],
                                         start=True, stop=True)
                        Br = mp.tile([128, 128], BF, tag="Br")
                        nc.vector.tensor_copy(out=Br[:], in_=pb[:])
                        pd2 = pbp.tile([128, 128], F32, space="PSUM", tag="pd2")
                        nc.tensor.matmul(out=pd2[:], lhsT=onsb[:],
                                         rhs=Dt[:, w0:w0 + 128],
                                         start=True, stop=True)
                        Dr = mp.tile([128, 128], BF, tag="Dr")
                        nc.vector.tensor_copy(out=Dr[:], in_=pd2[:])

                        psum = pmp.tile([128, RC], F32, space="PSUM", tag="ps")
                        gi = 0
                        for s in range(NS):
                            ga, _ = span[s]
                            for j in range(int(G[s][w])):
                                g = int(cumG[s][w]) - ga + j
                                gg = g
                                t2 = mp.tile([128, 128], BF, tag="t2")
                                nc.scalar.activation(
                                    t2[:], Dr[:], AF.Copy,
                                    scale=Cts[s][:, gg:gg + 1])
                                t1 = mp.tile([128, 128], BF, tag="t1")
                                nc.vector.scalar_tensor_tensor(
                                    out=t1[:], in0=Br[:],
                                    scalar=Ats[s][:, gg:gg + 1], in1=t2[:],
                                    op0=OP.mult, op1=OP.max)
                                sal = mp.tile([128, 128], BF, tag="sal")
                                nc.vector.scalar_tensor_tensor(
                                    out=sal[:], in0=iosb[:],
                                    scalar=dcs[s][:, gg:gg + 1], in1=t1[:],
                                    op0=OP.is_equal, op1=OP.mult)
                                nc.tensor.matmul(
                                    out=psum[:], lhsT=sal[:],
                                    rhs=gts[s][:, g, 0:RC],
                                    start=(gi == 0), stop=(gi == ngrp - 1))
                                gi += 1
                        # ---- window tail
                        pc = wp.tile([128, RC], F32, tag="pc")
                        nc.vector.tensor_copy(out=pc[:], in_=psum[:])
                        u = wp.tile([128, FH], F32, tag="u")
                        nc.vector.tensor_tensor(out=u[:], in0=pc[:, 0:FH],
                                                in1=pc[:, FH:2 * FH],
                                                op=OP.add)
                        zeps = wp.tile([128, 1], F32, tag="zeps")
                        nc.vector.tensor_scalar(
                            out=zeps[:], in0=pc[:, 2 * FH:2 * FH + 1],
                            scalar1=1e-16, scalar2=None, op0=OP.add)
                        rz = wp.tile([128, 1], F32, tag="rz")
                        nc.vector.reciprocal(out=rz[:], in_=zeps[:])
                        o1 = wp.tile([128, FH], F32, tag="o1")
                        nc.vector.tensor_scalar(
                            out=o1[:], in0=u[:], scalar1=rz[:], scalar2=None,
                            op0=OP.mult)
                        xb = wp.tile([128, FH], F32, tag="xb")
                        nc.vector.tensor_tensor(out=xb[:], in0=o1[:],
                                                in1=bsb[:], op=OP.add)
                        nr = min(WIN, CH - w0)
                        if layer == 1:
                            x2 = wp.tile([128, FH], F32, tag="x2")
                            nc.vector.tensor_scalar(
                                out=x2[:], in0=xb[:], scalar1=0.0,
                                scalar2=None, op0=OP.max)
                            pt = ptp.tile([128, 128], F32, space="PSUM",
                                          tag="pt")
                            nc.tensor.transpose(pt[:], x2[:], idn[:])
                            nc.vector.tensor_copy(out=x2T[:, w0:w0 + WIN],
                                                  in_=pt[:, 0:WIN])
                        else:
                            sg = wp.tile([128, FH], F32, tag="sg")
                            nc.scalar.activation(sg[:], xb[:], AF.Sigmoid)
                            nc.sync.dma_start(out=outp[w0:w0 + nr, :],
                                              in_=sg[0:nr, :])

            if layer == 1:
                with tc.tile_pool(name="tl", bufs=3) as tp, \
                     tc.tile_pool(name="tc1", bufs=1) as tcp, \
                     tc.tile_pool(name="ph2", bufs=2, space="PSUM") as php, \
                     tc.tile_pool(name="psv", bufs=2, space="PSUM") as psp:
                    h2T = tcp.tile([FO, WT], F32, tag="h2T")
                    CW = 512
                    for o in range(0, WT, CW):
                        cw = min(CW, WT - o)
                        ph = php.tile([FO, CW], F32, space="PSUM", tag="ph")
                        nc.tensor.matmul(out=ph[:, :cw], lhsT=w2sb[:],
                                         rhs=x2T[:, o:o + cw],
                                         start=True, stop=True)
                        nc.vector.tensor_copy(out=h2T[:, o:o + cw], in_=ph[:, :cw])
                        hh = tp.tile([FO, CW], BF, tag="hh")
                        nc.scalar.activation(hh[:, :cw], ph[:, :cw], AF.Copy)
                        tmp = tp.tile([FO, CW], F32, tag="tmp")
                        nc.vector.tensor_tensor(out=tmp[:, :cw], in0=ph[:, :cw],
                                                in1=hh[:, :cw], op=OP.subtract)
                        hl = tp.tile([FO, CW], BF, tag="hl")
                        nc.vector.tensor_copy(out=hl[:, :cw], in_=tmp[:, :cw])
                        nc.sync.dma_start(out=hhi[:, o:o + cw], in_=hh[:, :cw])
                        nc.sync.dma_start(out=hlo[:, o:o + cw], in_=hl[:, :cw])
                    for o in range(0, WT, CW):
                        cw = min(CW, WT - o)
                        ps = psp.tile([1, CW], F32, space="PSUM", tag="ps2")
                        nc.tensor.matmul(out=ps[:, :cw], lhsT=a2ssb[:],
                                         rhs=h2T[:, o:o + cw],
                                         start=True, stop=True)
                        sh = tp.tile([1, CW], BF, tag="sh")
                        nc.scalar.activation(sh[:, :cw], ps[:, :cw], AF.Copy)
                        tmp = tp.tile([1, CW], F32, tag="tms")
                        nc.vector.tensor_tensor(out=tmp[:, :cw], in0=ps[:, :cw],
                                                in1=sh[:, :cw], op=OP.subtract)
                        sl = tp.tile([1, CW], BF, tag="sl")
                        nc.vector.tensor_copy(out=sl[:, :cw], in_=tmp[:, :cw])
                        nc.sync.dma_start(out=shi[:, o:o + cw], in_=sh[:, :cw])
                        nc.sync.dma_start(out=slo[:, o:o + cw], in_=sl[:, :cw])
                        pd = psp.tile([1, CW], F32, space="PSUM", tag="pd")
                        nc.tensor.matmul(out=pd[:, :cw], lhsT=a2dsb[:],
                                         rhs=h2T[:, o:o + cw],
                                         start=True, stop=True)
                        dv = tp.tile([1, CW], F32, tag="dv")
                        nc.vector.tensor_copy(out=dv[:, :cw], in_=pd[:, :cw])
                        nc.sync.dma_start(out=d2o[:, o:o + cw], in_=dv[:, :cw])
    nc.finalize()
    return nc


# ------------------------------------------------------------------- driver
def kernel(edge_index, embed, W1, a_src1, a_dst1, b1, W2, a_src2, a_dst2, b2):
    N, C = embed.shape
    H = W1.shape[1]
    K = W2.shape[1]
    CH = N // NCORES
    meta = _preprocess(np.asarray(edge_index), N)
    NW, NS, G, cumG, nslot = (meta['NW'], meta['NS'], meta['G'],
                              meta['cumG'], meta['nslot'])
    WT = NW * WIN
    BW = WT + 128
    cores = list(range(NCORES))

    # ---- NEFF 1
    nc1 = _build_neff1(N, C, H, CH)
    maps1 = []
    for c in range(NCORES):
        xt = np.ascontiguousarray(embed[c * CH:(c + 1) * CH, :].T)
        maps1.append({"xT": xt.astype(np.float32),
                      "W1": np.asarray(W1, np.float32),
                      "a1s": np.asarray(a_src1, np.float32)[:, None],
                      "a1d": np.asarray(a_dst1, np.float32)[:, None]})
    print("[kernel] NEFF1 built, running...", file=sys.stderr, flush=True)
    r1 = run_bass_kernel_spmd(nc1, maps1, cores).results
    print("[kernel] NEFF1 done", file=sys.stderr, flush=True)

    T1 = np.zeros((N, 384), BF16)
    d1 = np.zeros((NCORES, 1, BW), np.float32)
    for c in range(NCORES):
        sl = slice(c * CH, (c + 1) * CH)
        T1[sl, 0:H] = r1[c]["hhi"].T
        T1[sl, H:2 * H] = r1[c]["hlo"].T
        T1[sl, 2 * H] = BF16(1.0)
        T1[sl, 2 * H + 1] = r1[c]["shi"][0]
        T1[sl, 2 * H + 2] = r1[c]["slo"][0]
        d1[c, 0, :CH] = r1[c]["d1o"][0]

    iota_np = np.tile(np.arange(128, dtype=np.float32), (128, 1)).astype(BF16)
    ones_np = np.ones((1, 128), BF16)

    # ---- NEFF 2
    nc2 = _build_edge_neff(N, CH, NW, NS, G, cumG, nslot, 1, H, K, None)
    maps2 = []
    for c in range(NCORES):
        m = {"T": T1, "dloc": d1[c], "iot": iota_np, "one1": ones_np,
             "brep": np.tile(np.asarray(b1, np.float32), (128, 1)),
             "W2": np.asarray(W2, np.float32),
             "a2s": np.asarray(a_src2, np.float32)[:, None],
             "a2d": np.asarray(a_dst2, np.float32)[:, None]}
        for s in range(NS):
            if nslot[s] == 0:
                continue
            m[f"gidx{s}"] = meta['cores'][c][0][s]
            m[f"dcol{s}"] = meta['cores'][c][1][s]
        maps2.append(m)
    print("[kernel] NEFF2 built, running...", file=sys.stderr, flush=True)
    r2 = run_bass_kernel_spmd(nc2, maps2, cores).results
    print("[kernel] NEFF2 done", file=sys.stderr, flush=True)

    T2 = np.zeros((N, 256), BF16)
    d2 = np.zeros((NCORES, 1, BW), np.float32)
    for c in range(NCORES):
        sl = slice(c * CH, (c + 1) * CH)
        T2[sl, 0:K] = r2[c]["hhi"][:, :CH].T
        T2[sl, K:2 * K] = r2[c]["hlo"][:, :CH].T
        T2[sl, 2 * K] = BF16(1.0)
        T2[sl, 2 * K + 1] = r2[c]["shi"][0, :CH]
        T2[sl, 2 * K + 2] = r2[c]["slo"][0, :CH]
        d2[c, 0, :CH] = r2[c]["d2o"][0, :CH]

    # ---- NEFF 3
    nc3 = _build_edge_neff(N, CH, NW, NS, G, cumG, nslot, 2, K, None, None)
    maps3 = []
    for c in range(NCORES):
        m = {"T": T2, "dloc": d2[c], "iot": iota_np, "one1": ones_np,
             "brep": np.tile(np.asarray(b2, np.float32), (128, 1))}
        for s in range(NS):
            if nslot[s] == 0:
                continue
            m[f"gidx{s}"] = meta['cores'][c][0][s]
            m[f"dcol{s}"] = meta['cores'][c][1][s]
        maps3.append(m)
    print("[kernel] NEFF3 built, running...", file=sys.stderr, flush=True)
    r3 = run_bass_kernel_spmd(nc3, maps3, cores).results
    print("[kernel] NEFF3 done", file=sys.stderr, flush=True)

    out = np.concatenate([r3[c]["out"] for c in range(NCORES)], axis=0)
    return out.astype(np.float32)



# revision 7
# speedup vs baseline: 1.4131x; 1.4131x over previous
"""2-layer GAT (PyG GATConv, heads=1) on 8 Trainium2 NeuronCores — v2.

Strategy (dst-owner sharding):
  - Nodes in 8 contiguous chunks of N/8; edges owned by dst's core.
  - 3 NEFF launches; host does data movement between them.
    NEFF#1: per-core h1 = embed_chunk @ W1, s1/d1 = h1 @ a_{src,dst}1 (bf16 h,
            f32 s/d) -> host builds gather table T1 [N,256]bf16 = [h|1|pad].
    NEFF#2: L1 edge phase: per-set continuous edge streams (sorted by dst
            window), dma_gather 512B rows in 2560-idx chunks; per-edge
            attention weight w_e = exp(LeakyReLU(s_src+d_dst)) from a
            host-prepared per-slot bf16 sum a_e (exp on ACT); indicator
            matrix sal[p,c] = w_p * 1[dcol_p == c] in ONE DVE tensor_scalar;
            TensorE: psum[dst,f|Z] += sal^T @ [h|1].  Self-loops are NOT in
            the streams; they are injected at the window tail from the local
            table slice (saves ~6% of gather descriptors).  Tail: out1 =
            (u + w_self*h_loc)/(Z + w_self + eps) + b1; relu; transpose into
            x2T; h2 = x2 @ W2; s2/d2.
    NEFF#3: same machinery on T2 [N,128]bf16 = [h2|1|pad] (256B rows),
            final sigmoid output.
  - Streams are continuous per (set = 32768 src rows): groups of 128 edge
    slots can span window boundaries (multiple matmul segments per group),
    eliminating per-(set,window) padding.  Only cross-core max-padding
    remains (slots with idx->row0, dcol=127 dummy).
"""
import os
import sys

if '/opt/trn_rl_repo' not in sys.path:
    sys.path.insert(0, '/opt/trn_rl_repo')

KBISECT = int(os.environ.get('KBISECT', '0'))  # 0=full 1=no tails/h2 2=no matmul

import numpy as np
import ml_dtypes

from concourse import bacc, mybir
import concourse.tile as tile
from concourse.bass_utils import run_bass_kernel_spmd
from concourse.masks import make_identity

BF16 = ml_dtypes.bfloat16
NCORES = 8
WIN = 127            # dsts per psum window (col/partition 127 = dummy)
CHUNK = 2560         # gather idxs per dma_gather (20 groups of 128)
GPC = CHUNK // 128
F32 = mybir.dt.float32
BF = mybir.dt.bfloat16
I16 = mybir.dt.int16
AF = mybir.ActivationFunctionType
OP = mybir.AluOpType


# ----------------------------------------------------------------- host pre
def _preprocess(edge_index, N):
    """Bucket real edges (no self-loops) by (dst core, src set, dst window).

    Streams are per (core, set), window-sorted, padded per (set,window) to the
    max count across cores so all cores share one instruction stream.
    """
    CH = N // NCORES
    NW = -(-CH // WIN)
    SETROWS = min(32768, N)
    NS = -(-N // SETROWS)
    src = edge_index[0].astype(np.int64)
    dst = edge_index[1].astype(np.int64)
    owner = dst // CH
    dl = dst - owner * CH
    w = dl // WIN
    col = dl - w * WIN
    sid = src // SETROWS
    NB = NS * NW
    key = sid * NW + w

    cnt = np.zeros((NCORES, NB), np.int64)
    for c in range(NCORES):
        cnt[c] = np.bincount(key[owner == c], minlength=NB)
    mx = cnt.max(axis=0).reshape(NS, NW)
    off = np.zeros((NS, NW + 1), np.int64)
    off[:, 1:] = np.cumsum(mx, axis=1)
    Ls = off[:, -1]
    nchunk = -(-Ls // CHUNK)
    Lpad = nchunk * CHUNK

    # segments ordered by (window, set, group)
    segs = []
    jbase = np.full((NS, NW), -1, np.int64)
    g0arr = np.zeros((NS, NW), np.int64)
    for w_ in range(NW):
        for s in range(NS):
            a, b = off[s, w_], off[s, w_ + 1]
            if b == a:
                continue
            g0, g1 = a // 128, (b - 1) // 128
            jbase[s, w_] = len(segs)
            g0arr[s, w_] = g0
            for g in range(g0, g1 + 1):
                segs.append((s, g // GPC, g % GPC, w_))
    NSEG = len(segs)

    cores = []
    for c in range(NCORES):
        m = owner == c
        es, ed, ekey = src[m], dst[m], key[m]
        ecol, ew, esid = col[m], w[m], sid[m]
        o = np.argsort(ekey, kind='stable')
        es, ed, ekey, ecol, ew, esid = (es[o], ed[o], ekey[o], ecol[o],
                                        ew[o], esid[o])
        cc = np.zeros(NB + 1, np.int64)
        cc[1:] = np.cumsum(np.bincount(ekey, minlength=NB))
        rank = np.arange(len(ekey)) - cc[ekey]
        slot = off[esid, ew] + rank

        slot_src, slot_dst, idx16, gidx = [], [], [], []
        for s in range(NS):
            ss = np.full(Lpad[s], -1, np.int64)
            sd = np.full(Lpad[s], -1, np.int64)
            ms = esid == s
            ss[slot[ms]] = es[ms]
            sd[slot[ms]] = ed[ms]
            slot_src.append(ss)
            slot_dst.append(sd)
            ix = (ss - s * SETROWS).astype(np.int64)
            ix[ss < 0] = 0                 # mid-stream pads -> row 0 (masked)
            ix[Ls[s]:] = -1                # trailing pads -> dropped by HW
            ix = ix.astype(np.int16)
            idx16.append(ix)
            # wrap per chunk: [128, nchunk*CHUNK/16]
            blocks = []
            for cch in range(nchunk[s]):
                blk = ix[cch * CHUNK:(cch + 1) * CHUNK]
                blocks.append(np.tile(blk.reshape(-1, 16).T, (8, 1)))
            gidx.append(np.ascontiguousarray(np.concatenate(blocks, axis=1))
                        if blocks else np.zeros((128, 0), np.int16))

        dcol = np.full((128, max(NSEG, 1)), 127.0, np.float32)
        g = slot // 128
        j = jbase[esid, ew] + g - g0arr[esid, ew]
        p = slot % 128
        dcol[p, j] = ecol
        cores.append(dict(slot_src=slot_src, slot_dst=slot_dst,
                          gidx=gidx, dcol=np.ascontiguousarray(dcol)))
    return dict(CH=CH, NW=NW, NS=NS, SETROWS=SETROWS, off=off, Ls=Ls,
                nchunk=nchunk, Lpad=Lpad, segs=segs, NSEG=max(NSEG, 1),
                cores=cores)


# ------------------------------------------------------------------ NEFF #1
def _build_neff1(N, C, H, CH):
    nc = bacc.Bacc(None, target_bir_lowering=False)
    xT = nc.declare_dram_parameter("xT", [C, CH], F32, isOutput=False)
    W1 = nc.declare_dram_parameter("W1", [C, H], F32, isOutput=False)
    a1s = nc.declare_dram_parameter("a1s", [H, 1], F32, isOutput=False)
    a1d = nc.declare_dram_parameter("a1d", [H, 1], F32, isOutput=False)
    h1o = nc.declare_dram_parameter("h1o", [H, CH], BF, isOutput=True)
    s1o = nc.declare_dram_parameter("s1o", [1, CH], F32, isOutput=True)
    d1o = nc.declare_dram_parameter("d1o", [1, CH], F32, isOutput=True)

    KT = -(-C // 128)
    with tile.TileContext(nc) as tc:
        with tc.tile_pool(name="cst", bufs=1) as cp, \
             tc.tile_pool(name="wk", bufs=3) as wp, \
             tc.tile_pool(name="ps", bufs=2, space="PSUM") as pp, \
             tc.tile_pool(name="ps1", bufs=2, space="PSUM") as pp1:
            xts, w1s = [], []
            for k in range(KT):
                kc = min(128, C - 128 * k)
                xt = cp.tile([kc, CH], F32, tag=f"xt{k}")
                nc.sync.dma_start(out=xt[:], in_=xT[128 * k:128 * k + kc, :])
                w1 = cp.tile([kc, H], F32, tag=f"w1{k}")
                nc.sync.dma_start(out=w1[:], in_=W1[128 * k:128 * k + kc, :])
                xts.append(xt)
                w1s.append(w1)
            asb = cp.tile([H, 1], F32, tag="a1s")
            nc.sync.dma_start(out=asb[:], in_=a1s[:])
            adb = cp.tile([H, 1], F32, tag="a1d")
            nc.sync.dma_start(out=adb[:], in_=a1d[:])
            h1T = cp.tile([H, CH], F32, tag="h1T")

            CW = 500
            for o in range(0, CH, CW):
                cw = min(CW, CH - o)
                ph = pp.tile([H, CW], F32, space="PSUM", tag="ph")
                for k in range(KT):
                    nc.tensor.matmul(out=ph[:, :cw], lhsT=w1s[k][:],
                                     rhs=xts[k][:, o:o + cw],
                                     start=(k == 0), stop=(k == KT - 1))
                nc.vector.tensor_copy(out=h1T[:, o:o + cw], in_=ph[:, :cw])
                hb = wp.tile([H, CW], BF, tag="hb")
                nc.scalar.activation(hb[:, :cw], ph[:, :cw], AF.Copy)
                nc.sync.dma_start(out=h1o[:, o:o + cw], in_=hb[:, :cw])
            for o in range(0, CH, CW):
                cw = min(CW, CH - o)
                ps = pp1.tile([1, CW], F32, space="PSUM", tag="psv")
                nc.tensor.matmul(out=ps[:, :cw], lhsT=asb[:],
                                 rhs=h1T[:, o:o + cw], start=True, stop=True)
                sv = wp.tile([1, CW], F32, tag="sv")
                nc.vector.tensor_copy(out=sv[:, :cw], in_=ps[:, :cw])
                nc.sync.dma_start(out=s1o[:, o:o + cw], in_=sv[:, :cw])
                pd = pp1.tile([1, CW], F32, space="PSUM", tag="pdv")
                nc.tensor.matmul(out=pd[:, :cw], lhsT=adb[:],
                                 rhs=h1T[:, o:o + cw], start=True, stop=True)
                dv = wp.tile([1, CW], F32, tag="dv")
                nc.vector.tensor_copy(out=dv[:, :cw], in_=pd[:, :cw])
                nc.sync.dma_start(out=d1o[:, o:o + cw], in_=dv[:, :cw])
    nc.finalize()
    return nc


# --------------------------------------------------------- edge-phase NEFFs
def _build_edge_neff(N, meta, layer, FH, FO):
    """layer 1: FH=128 (h1), FO=64 -> emits h2/s2/d2.  layer 2: FH=64, sigmoid out.

    Table rows: L1 [h(128)|1|pad] 256 bf16 cols (512B); L2 [h2(64)|1|pad] 128
    cols (256B).  RC = FH+1 rhs cols.
    """
    CH, NW, NS = meta['CH'], meta['NW'], meta['NS']
    SETROWS, nchunk, segs = meta['SETROWS'], meta['nchunk'], meta['segs']
    NSEG, Lpad = meta['NSEG'], meta['Lpad']
    ELEM = 256 if layer == 1 else 128
    RC = FH + 1

    nc = bacc.Bacc(None, target_bir_lowering=False)
    T = nc.declare_dram_parameter("T", [N, ELEM], BF, isOutput=False)
    Tloc = nc.declare_dram_parameter("Tloc", [CH, ELEM], BF, isOutput=False)
    dcolp = nc.declare_dram_parameter("dcol", [128, NSEG], F32, isOutput=False)
    aselfp = nc.declare_dram_parameter("aself", [128, NW], BF, isOutput=False)
    browp = nc.declare_dram_parameter("brow", [128, FH], F32, isOutput=False)
    iotp = nc.declare_dram_parameter("iot", [128, 128], BF, isOutput=False)
    gidxp, acolp = [], []
    for s in range(NS):
        ng = Lpad[s] // 128
        if nchunk[s] == 0:
            gidxp.append(None)
            acolp.append(None)
            continue
        gidxp.append(nc.declare_dram_parameter(
            f"gidx{s}", [128, (nchunk[s] * CHUNK) // 16], I16, isOutput=False))
        acolp.append(nc.declare_dram_parameter(
            f"acol{s}", [128, ng], BF, isOutput=False))
    if layer == 1:
        W2p = nc.declare_dram_parameter("W2", [FH, FO], BF, isOutput=False)
        a2sp = nc.declare_dram_parameter("a2s", [FO, 1], F32, isOutput=False)
        a2dp = nc.declare_dram_parameter("a2d", [FO, 1], F32, isOutput=False)
        h2o = nc.declare_dram_parameter("h2o", [FO, CH], BF, isOutput=True)
        s2o = nc.declare_dram_parameter("s2o", [1, CH], F32, isOutput=True)
        d2o = nc.declare_dram_parameter("d2o", [1, CH], F32, isOutput=True)
    else:
        outp = nc.declare_dram_parameter("out", [CH, FH], F32, isOutput=True)

    from collections import defaultdict
    byw = defaultdict(list)
    for j, (s, cch, gl, w_) in enumerate(segs):
        byw[w_].append((j, s, cch, gl))

    with tile.TileContext(nc) as tc:
        with tc.tile_pool(name="cst", bufs=1) as cp:
            iosb = cp.tile([128, 128], BF, tag="io")
            nc.sync.dma_start(out=iosb[:], in_=iotp[:])
            brow = cp.tile([128, FH], F32, tag="br")
            nc.sync.dma_start(out=brow[:], in_=browp[:])
            dcsb = cp.tile([128, NSEG], F32, tag="dc")
            nc.sync.dma_start(out=dcsb[:], in_=dcolp[:])
            epsc = cp.tile([128, 1], F32, tag="eps")
            nc.vector.memset(epsc[:], 1e-16)
            # self-loop weights
            aself = cp.tile([128, NW], BF, tag="asf")
            nc.sync.dma_start(out=aself[:], in_=aselfp[:])
            wself = cp.tile([128, NW], F32, tag="wsf")
            with tc.tile_pool(name="tmpw", bufs=1) as twp:
                eA = twp.tile([128, NW], F32, tag="eA")
                nc.scalar.activation(eA[:], aself[:], AF.Exp)
                eC = twp.tile([128, NW], F32, tag="eC")
                nc.scalar.activation(eC[:], aself[:], AF.Exp, scale=0.2)
                nc.vector.tensor_tensor(out=wself[:], in0=eA[:], in1=eC[:],
                                        op=OP.max)
            # per-set edge weights
            wvs = []
            for s in range(NS):
                if acolp[s] is None:
                    wvs.append(None)
                    continue
                ng = Lpad[s] // 128
                ac = cp.tile([128, ng], BF, tag=f"ac{s}")
                nc.sync.dma_start(out=ac[:], in_=acolp[s][:])
                wv = cp.tile([128, ng], F32, tag=f"wv{s}")
                with tc.tile_pool(name=f"tw{s}", bufs=1) as twp:
                    eA = twp.tile([128, ng], F32, tag="eA")
                    nc.scalar.activation(eA[:], ac[:], AF.Exp)
                    eC = twp.tile([128, ng], F32, tag="eC")
                    nc.scalar.activation(eC[:], ac[:], AF.Exp, scale=0.2)
                    nc.vector.tensor_tensor(out=wv[:], in0=eA[:], in1=eC[:],
                                            op=OP.max)
                wvs.append(wv)
            if layer == 1:
                idn = cp.tile([128, 128], F32, tag="idn")
                make_identity(nc, idn[:])
                x2T = cp.tile([128, CH], BF, tag="x2T")
                w2sb = cp.tile([FH, FO], BF, tag="w2")
                nc.sync.dma_start(out=w2sb[:], in_=W2p[:])
                a2ssb = cp.tile([FO, 1], F32, tag="a2s")
                nc.sync.dma_start(out=a2ssb[:], in_=a2sp[:])
                a2dsb = cp.tile([FO, 1], F32, tag="a2d")
                nc.sync.dma_start(out=a2dsb[:], in_=a2dp[:])

            spools, ipools = [], []
            for s in range(NS):
                spools.append(tc.tile_pool(name=f"sp{s}", bufs=3))
                ipools.append(tc.tile_pool(name=f"ip{s}", bufs=3))
            import contextlib
            with contextlib.ExitStack() as stk:
                sp = [stk.enter_context(p) for p in spools]
                ip = [stk.enter_context(p) for p in ipools]
                mp = stk.enter_context(tc.tile_pool(name="msk", bufs=4))
                wp = stk.enter_context(tc.tile_pool(name="wk", bufs=3))
                hp = stk.enter_context(tc.tile_pool(name="hl", bufs=2))
                pmp = stk.enter_context(
                    tc.tile_pool(name="pm", bufs=2, space="PSUM"))
                ptp = stk.enter_context(
                    tc.tile_pool(name="pt", bufs=2, space="PSUM"))

                issued = [0] * NS
                strips = {}

                Ls = meta['Ls']

                def ensure_chunk(s, cneed):
                    tgt = min(cneed + 1, nchunk[s] - 1)
                    while issued[s] <= tgt:
                        c = issued[s]
                        nval = int(min(CHUNK, Ls[s] - c * CHUNK))
                        ix = ip[s].tile([128, CHUNK // 16], I16, tag=f"ix{s}")
                        nc.sync.dma_start(
                            out=ix[:],
                            in_=gidxp[s][:, c * (CHUNK // 16):
                                         (c + 1) * (CHUNK // 16)])
                        st = sp[s].tile([128, GPC, ELEM], BF, tag=f"st{s}")
                        nc.gpsimd.dma_gather(
                            out_ap=st[:],
                            in_ap=T[s * SETROWS:, :],
                            idxs_ap=ix[:],
                            num_idxs=CHUNK,
                            num_idxs_reg=nval,
                            elem_size=ELEM,
                            single_packet=False,
                        )
                        strips[(s, c)] = st
                        issued[s] += 1

                for w_ in range(NW):
                    nr = min(WIN, CH - w_ * WIN)
                    lst = byw.get(w_, [])
                    ps = pmp.tile([128, RC], F32, space="PSUM", tag="psw")
                    for i, (j, s, cch, gl) in enumerate(lst):
                        ensure_chunk(s, cch)
                        sal = mp.tile([128, 128], BF, tag="sal")
                        nc.vector.tensor_scalar(
                            out=sal[:], in0=iosb[:],
                            scalar1=dcsb[:, j:j + 1],
                            scalar2=wvs[s][:, cch * GPC + gl:
                                           cch * GPC + gl + 1],
                            op0=OP.is_equal, op1=OP.mult)
                        if KBISECT < 2:
                            nc.tensor.matmul(
                                out=ps[:], lhsT=sal[:],
                                rhs=strips[(s, cch)][:, gl, 0:RC],
                                start=(i == 0), stop=(i == len(lst) - 1))
                    pc = wp.tile([128, RC], F32, tag="pc")
                    if lst and KBISECT < 2:
                        nc.vector.tensor_copy(out=pc[:], in_=ps[:])
                    else:
                        nc.vector.memset(pc[:], 0.0)
                    if KBISECT >= 1:
                        continue
                    # self-loop injection + normalize + bias
                    hl = hp.tile([128, ELEM], BF, tag="hl")
                    nc.sync.dma_start(out=hl[0:nr, :],
                                      in_=Tloc[w_ * WIN:w_ * WIN + nr, :])
                    t1 = wp.tile([128, FH], F32, tag="t1")
                    nc.vector.tensor_scalar(out=t1[:], in0=hl[:, 0:FH],
                                            scalar1=wself[:, w_:w_ + 1],
                                            scalar2=None, op0=OP.mult)
                    u2 = wp.tile([128, FH], F32, tag="u2")
                    nc.vector.tensor_tensor(out=u2[:], in0=pc[:, 0:FH],
                                            in1=t1[:], op=OP.add)
                    z2 = wp.tile([128, 1], F32, tag="z2")
                    nc.vector.scalar_tensor_tensor(
                        out=z2[:], in0=pc[:, FH:FH + 1],
                        scalar=wself[:, w_:w_ + 1], in1=epsc[:],
                        op0=OP.add, op1=OP.add)
                    rz = wp.tile([128, 1], F32, tag="rz")
                    nc.vector.reciprocal(out=rz[:], in_=z2[:])
                    o1 = wp.tile([128, FH], F32, tag="o1")
                    nc.vector.tensor_scalar(out=o1[:], in0=u2[:],
                                            scalar1=rz[:], scalar2=None,
                                            op0=OP.mult)
                    xb = wp.tile([128, FH], F32, tag="xb")
                    nc.vector.tensor_tensor(out=xb[:], in0=o1[:], in1=brow[:],
                                            op=OP.add)
                    if layer == 1:
                        x2f = wp.tile([128, FH], F32, tag="x2f")
                        nc.vector.tensor_scalar(out=x2f[:], in0=xb[:],
                                                scalar1=0.0, scalar2=None,
                                                op0=OP.max)
                        pt = ptp.tile([128, 128], F32, space="PSUM", tag="pt")
                        nc.tensor.transpose(pt[:], x2f[:], idn[:])
                        nc.vector.tensor_copy(
                            out=x2T[:, w_ * WIN:w_ * WIN + nr],
                            in_=pt[:, 0:nr])
                    else:
                        sg = wp.tile([128, FH], F32, tag="sg")
                        nc.scalar.activation(sg[:], xb[:], AF.Sigmoid)
                        nc.sync.dma_start(out=outp[w_ * WIN:w_ * WIN + nr, :],
                                          in_=sg[0:nr, :])

            if layer == 1 and KBISECT == 0:
                with tc.tile_pool(name="tl", bufs=3) as tp, \
                     tc.tile_pool(name="ph2", bufs=2, space="PSUM") as php, \
                     tc.tile_pool(name="ps2", bufs=2, space="PSUM") as psp:
                    CW = 512
                    for o in range(0, CH, CW):
                        cw = min(CW, CH - o)
                        ph = php.tile([FO, CW], F32, space="PSUM", tag="ph")
                        nc.tensor.matmul(out=ph[:, :cw], lhsT=w2sb[:],
                                         rhs=x2T[:, o:o + cw],
                                         start=True, stop=True)
                        h2f = tp.tile([FO, CW], F32, tag="h2f")
                        nc.vector.tensor_copy(out=h2f[:, :cw], in_=ph[:, :cw])
                        h2b = tp.tile([FO, CW], BF, tag="h2b")
                        nc.scalar.activation(h2b[:, :cw], ph[:, :cw], AF.Copy)
                        nc.sync.dma_start(out=h2o[:, o:o + cw],
                                          in_=h2b[:, :cw])
                        ps2 = psp.tile([1, CW], F32, space="PSUM", tag="pss")
                        nc.tensor.matmul(out=ps2[:, :cw], lhsT=a2ssb[:],
                                         rhs=h2f[:, :cw], start=True,
                                         stop=True)
                        sv = tp.tile([1, CW], F32, tag="sv")
                        nc.vector.tensor_copy(out=sv[:, :cw], in_=ps2[:, :cw])
                        nc.sync.dma_start(out=s2o[:, o:o + cw], in_=sv[:, :cw])
                        pd2 = psp.tile([1, CW], F32, space="PSUM", tag="pdd")
                        nc.tensor.matmul(out=pd2[:, :cw], lhsT=a2dsb[:],
                                         rhs=h2f[:, :cw], start=True,
                                         stop=True)
                        dv = tp.tile([1, CW], F32, tag="dv")
                        nc.vector.tensor_copy(out=dv[:, :cw], in_=pd2[:, :cw])
                        nc.sync.dma_start(out=d2o[:, o:o + cw], in_=dv[:, :cw])
    nc.finalize()
    return nc


# ------------------------------------------------------------------- driver
def _acol_for(meta, c, s, sarr, darr):
    ss = meta['cores'][c]['slot_src'][s]
    sd = meta['cores'][c]['slot_dst'][s]
    a = np.zeros(len(ss), np.float32)
    m = ss >= 0
    a[m] = sarr[ss[m]] + darr[sd[m]]
    return np.ascontiguousarray(a.reshape(-1, 128).T.astype(BF16))


def _aself_for(meta, c, sarr, darr, N):
    CH, NW = meta['CH'], meta['NW']
    out = np.full((128, NW), -60.0, np.float32)
    for w_ in range(NW):
        nr = min(WIN, CH - w_ * WIN)
        i0 = c * CH + w_ * WIN
        out[:nr, w_] = sarr[i0:i0 + nr] + darr[i0:i0 + nr]
    return np.ascontiguousarray(out.astype(BF16))


def kernel(edge_index, embed, W1, a_src1, a_dst1, b1, W2, a_src2, a_dst2, b2):
    N, C = embed.shape
    H = W1.shape[1]
    K = W2.shape[1]
    CH = N // NCORES
    meta = _preprocess(np.asarray(edge_index), N)
    NS, NW = meta['NS'], meta['NW']
    cores = list(range(NCORES))
    iota_np = np.tile(np.arange(128, dtype=np.float32), (128, 1)).astype(BF16)

    # ---- NEFF 1
    nc1 = _build_neff1(N, C, H, CH)
    maps1 = []
    for c in range(NCORES):
        xt = np.ascontiguousarray(embed[c * CH:(c + 1) * CH, :].T)
        maps1.append({"xT": xt.astype(np.float32),
                      "W1": np.asarray(W1, np.float32),
                      "a1s": np.asarray(a_src1, np.float32)[:, None],
                      "a1d": np.asarray(a_dst1, np.float32)[:, None]})
    print("[kernel] NEFF1 built, running...", file=sys.stderr, flush=True)
    r1 = run_bass_kernel_spmd(nc1, maps1, cores).results
    print("[kernel] NEFF1 done", file=sys.stderr, flush=True)

    T1 = np.zeros((N, 256), BF16)
    s1 = np.zeros(N, np.float32)
    d1 = np.zeros(N, np.float32)
    for c in range(NCORES):
        sl = slice(c * CH, (c + 1) * CH)
        T1[sl, 0:H] = r1[c]["h1o"].T
        s1[sl] = r1[c]["s1o"][0]
        d1[sl] = r1[c]["d1o"][0]
    T1[:, H] = BF16(1.0)

    # ---- NEFF 2 (edge layer 1)
    nc2 = _build_edge_neff(N, meta, 1, H, K)
    maps2 = []
    for c in range(NCORES):
        m = {"T": T1, "Tloc": np.ascontiguousarray(T1[c * CH:(c + 1) * CH]),
             "dcol": meta['cores'][c]['dcol'],
             "aself": _aself_for(meta, c, s1, d1, N),
             "brow": np.tile(np.asarray(b1, np.float32), (128, 1)),
             "iot": iota_np,
             "W2": np.asarray(W2, np.float32).astype(BF16),
             "a2s": np.asarray(a_src2, np.float32)[:, None],
             "a2d": np.asarray(a_dst2, np.float32)[:, None]}
        for s in range(NS):
            if meta['nchunk'][s] == 0:
                continue
            m[f"gidx{s}"] = meta['cores'][c]['gidx'][s]
            m[f"acol{s}"] = _acol_for(meta, c, s, s1, d1)
        maps2.append(m)
    print("[kernel] NEFF2 built, running...", file=sys.stderr, flush=True)
    r2 = run_bass_kernel_spmd(nc2, maps2, cores).results
    print("[kernel] NEFF2 done", file=sys.stderr, flush=True)

    T2 = np.zeros((N, 128), BF16)
    s2 = np.zeros(N, np.float32)
    d2 = np.zeros(N, np.float32)
    for c in range(NCORES):
        sl = slice(c * CH, (c + 1) * CH)
        T2[sl, 0:K] = r2[c]["h2o"].T
        s2[sl] = r2[c]["s2o"][0]
        d2[sl] = r2[c]["d2o"][0]
    T2[:, K] = BF16(1.0)

    # ---- NEFF 3 (edge layer 2)
    nc3 = _build_edge_neff(N, meta, 2, K, None)
    maps3 = []
    for c in range(NCORES):
        m = {"T": T2, "Tloc": np.ascontiguousarray(T2[c * CH:(c + 1) * CH]),
             "dcol": meta['cores'][c]['dcol'],
             "aself": _aself_for(meta, c, s2, d2, N),
             "brow": np.tile(np.asarray(b2, np.float32), (128, 1)),
             "iot": iota_np}
        for s in range(NS):
            if meta['nchunk'][s] == 0:
                continue
            m[f"gidx{s}"] = meta['cores'][c]['gidx'][s]
            m[f"acol{s}"] = _acol_for(meta, c, s, s2, d2)
        maps3.append(m)
    print("[kernel] NEFF3 built, running...", file=sys.stderr, flush=True)
    r3 = run_bass_kernel_spmd(nc3, maps3, cores).results
    print("[kernel] NEFF3 done", file=sys.stderr, flush=True)

    out = np.concatenate([r3[c]["out"] for c in range(NCORES)], axis=0)
    return out.astype(np.float32)
